# revision 28
# baseline (speedup 1.0000x reference)
"""Trainium2 Bass kernel for a GQA attention block (B=4, S=1024, D=2048,
NH=32, NKV=8, HD=64) with RoPE, causal mask, and output projection.

Sharding: 8 cores = 4 batches x 2 head-groups (tensor-parallel over heads).
Each core computes, for its (batch b, head-group hg):
  q/k/v projections (contracted over D), RoPE, attention for 16 q-heads /
  4 kv-heads, and a partial out = attn_out @ wo[:, hg].T.
Host sums the two partial outputs per batch and assembles the (repeated)
k/v caches from the per-core unique kv heads.

All matmuls run as float32r (fp32 storage, fast PE path; ~1cyc/row at
K=128). Attention uses the transposed-scores layout scores_T[k, q]:
softmax normalization (sum over k) falls out of the PV matmul via an
extra ones-column in V, and exp() is a single ACT pass PSUM->SBUF.
Scores contract over HD=64 only, which is LDWEIGHTS-bound at K=64, so
kT stores each kv head zero-padded to K=128 in two half-aligned
variants and both heads of a pair share one [128, 512] scores psum /
exp tile (halves the ACT instruction count).
"""

import numpy as np
from contextlib import ExitStack

import concourse.bass as bass
import concourse.mybir as mybir
import concourse.tile as tile
from concourse import bacc
from concourse.bass_utils import run_bass_kernel_spmd
from concourse.masks import make_identity

F32 = mybir.dt.float32
F32R = mybir.dt.float32r

B, S, D = 4, 1024, 2048
NH, NKV, HD = 32, 8, 64
NREP = NH // NKV
P = 128
NT = S // P            # 8 s-tiles of 128
QH = NH // 2           # 16 q heads per core
KVH = NKV // 2         # 4 kv heads per core
NKD = D // P           # 16 contraction tiles over D
QCH = 4                # q chunks of 256 for attention
QC = S // QCH          # 256
SCALE = 1.0 / np.sqrt(HD)  # 0.125


def build_nc(mode):
    """mode: 'causal' (block-skip + diag tri mask), 'zeros' (dense, no mask),
    'mask' (dense + full additive mask)."""
    nc = bacc.Bacc("TRN2", target_bir_lowering=False, debug=False)

    xT = nc.dram_tensor("xT", [D, S], F32R, kind="ExternalInput")
    wqT = nc.dram_tensor("wqT", [D, QH * HD], F32R, kind="ExternalInput")
    wkvT = nc.dram_tensor("wkvT", [D, 2 * KVH * HD], F32R, kind="ExternalInput")
    woT = nc.dram_tensor("woT", [QH * HD, D], F32R, kind="ExternalInput")
    cosf = nc.dram_tensor("cosf", [S, HD // 2], F32, kind="ExternalInput")
    sinf = nc.dram_tensor("sinf", [S, HD // 2], F32, kind="ExternalInput")
    # q-rope runs in [hd-part, S-free] layout: cos replicated on all four
    # 32-row blocks; sin sign-flipped on t0 blocks (o = q*cos + qswap*sinS)
    cosT4 = nc.dram_tensor("cosT4", [P, S], F32, kind="ExternalInput")
    sinS = nc.dram_tensor("sinS", [P, S], F32, kind="ExternalInput")
    if mode == "causal":
        trimask = nc.dram_tensor("trimask", [P, P], F32, kind="ExternalInput")
    if mode == "mask":
        maskT = nc.dram_tensor("maskT", [S, S], F32, kind="ExternalInput")

    outp = nc.dram_tensor("outp", [S, D], F32, kind="ExternalOutput")
    ko = nc.dram_tensor("ko", [KVH, S, HD], F32, kind="ExternalOutput")
    vo = nc.dram_tensor("vo", [KVH, S, HD], F32, kind="ExternalOutput")

    with tile.TileContext(nc) as tc, ExitStack() as top:
        constp = top.enter_context(tc.tile_pool(name="const", bufs=1))
        ident = constp.tile([P, P], F32)
        make_identity(nc, ident[:])
        cos_sb = constp.tile([P, NT, HD // 2], F32)
        sin_sb = constp.tile([P, NT, HD // 2], F32)
        nc.sync.dma_start(cos_sb[:], cosf.rearrange("(t p) f -> p t f", p=P))
        nc.sync.dma_start(sin_sb[:], sinf.rearrange("(t p) f -> p t f", p=P))
        cosT4_sb = constp.tile([P, S], F32)
        sinS_sb = constp.tile([P, S], F32)
        nc.sync.dma_start(cosT4_sb[:], cosT4[:])
        nc.sync.dma_start(sinS_sb[:], sinS[:])
        if mode == "causal":
            tri_sb = constp.tile([P, P], F32)
            nc.sync.dma_start(tri_sb[:], trimask[:])
        if mode == "mask":
            maskT_sb = constp.tile([P, NT, S], F32)
            nc.sync.dma_start(maskT_sb[:], maskT.rearrange("(t p) q -> p t q", p=P))

        # persistent across phases: qT/kT (transposed, rope'd), v (+ones col)
        persist = top.enter_context(tc.tile_pool(name="persist", bufs=1))
        qT = persist.tile([P, QH // 2, S], F32R)      # block jj: heads 2jj,2jj+1
        # kT holds each kv head zero-padded to K=128 in two variants:
        # variant 0 = [k_g | 0] pairs with the q-half on partitions 0:64,
        # variant 1 = [0 | k_g] pairs with the q-half on partitions 64:128.
        # This lets every scores matmul run with full-128 contraction (the
        # K=64 form is ~2x slower, LDWEIGHTS-bound) and full-partition APs.
        kT = persist.tile([P, KVH, 2, S], F32R)
        nc.vector.memset(kT[:].bitcast(F32), 0.0)
        vsb = persist.tile([P, NT, KVH, HD + 1], F32R)
        nc.vector.memset(vsb[:, :, :, HD].bitcast(F32), 1.0)

        # ---------------- phase 1: projections + rope + transposes ----------
        with ExitStack() as ph1:
            xp = ph1.enter_context(tc.tile_pool(name="xpool", bufs=1))
            rp = ph1.enter_context(tc.tile_pool(name="rope", bufs=3))
            pp = ph1.enter_context(tc.tile_pool(name="projps", bufs=2, space="PSUM"))
            tp = ph1.enter_context(tc.tile_pool(name="trps", bufs=3, space="PSUM"))

            xT_sb = xp.tile([P, NKD, S], F32R)
            xTr = xT.rearrange("(o p) s -> p o s", p=P)
            for xq4 in range(4):
                nc.sync.dma_start(
                    xT_sb[:, 4 * xq4:4 * (xq4 + 1), :],
                    xTr[:, 4 * xq4:4 * (xq4 + 1), :])

            def rope(dst, src, t, nh):
                """dst, src: [P, nh, HD] APs (dst SBUF, src PSUM), s-tile t."""
                t0 = src[:, :, 0::2]
                t1 = src[:, :, 1::2]
                r0 = dst[:, :, 0::2]
                r1 = dst[:, :, 1::2]
                cosb = cos_sb[:, t, None, :].to_broadcast((P, nh, HD // 2))
                sinb = sin_sb[:, t, None, :].to_broadcast((P, nh, HD // 2))
                tmp = rp.tile([P, nh, HD // 2], F32, tag=f"ropetmp{nh}")
                M = mybir.AluOpType.mult
                nc.vector.tensor_tensor(out=tmp[:], in0=t1, in1=sinb, op=M)
                nc.vector.tensor_tensor(out=r0, in0=t0, in1=cosb, op=M)
                nc.vector.tensor_sub(out=r0, in0=r0, in1=tmp[:])
                nc.vector.tensor_tensor(out=tmp[:], in0=t0, in1=sinb, op=M)
                nc.vector.tensor_tensor(out=r1, in0=t1, in1=cosb, op=M)
                nc.vector.tensor_add(out=r1, in0=r1, in1=tmp[:])

            # K/V projection (k cols 0:256, v cols 256:512 of wkv)
            with ExitStack() as phkv:
                wkvp = phkv.enter_context(tc.tile_pool(name="wkvpool", bufs=1))
                wkv_sb = wkvp.tile([P, NKD, 2 * KVH * HD], F32R)
                nc.scalar.dma_start(wkv_sb[:], wkvT.rearrange("(o p) n -> p o n", p=P))
                for t in range(NT):
                    ps = pp.tile([P, 2, KVH, HD], F32, tag="kvps")
                    for kd in range(NKD):
                        nc.tensor.matmul(
                            ps[:],
                            lhsT=xT_sb[:, kd, t * P:(t + 1) * P],
                            rhs=wkv_sb[:, kd, :],
                            start=(kd == 0),
                            stop=(kd == NKD - 1),
                        )
                    # V: copy into vsb (ones col pre-set), DMA out
                    nc.vector.tensor_copy(out=vsb[:, t, :, 0:HD], in_=ps[:, 1])
                    for g in range(KVH):
                        nc.gpsimd.dma_start(
                            vo[g, t * P:(t + 1) * P, :], vsb[:, t, g, 0:HD].bitcast(F32))
                    # K: rope, DMA out, transpose into kT
                    kr = rp.tile([P, KVH, HD], F32, tag="krope")
                    rope(kr[:], ps[:, 0], t, KVH)
                    for g in range(KVH):
                        nc.gpsimd.dma_start(ko[g, t * P:(t + 1) * P, :], kr[:, g, :])
                    ts_ = slice(t * P, (t + 1) * P)
                    # deinterleave hd pairs into t0|t1 blocks to match the
                    # permuted q layout (scores are permutation-invariant)
                    kr2 = rp.tile([P, KVH, HD], F32, tag="krope2")
                    nc.vector.tensor_copy(
                        out=kr2[:, :, 0:HD // 2], in_=kr[:, :, 0::2])
                    nc.vector.tensor_copy(
                        out=kr2[:, :, HD // 2:HD], in_=kr[:, :, 1::2])
                    for gg in range(KVH // 2):
                        trp = tp.tile([P, P], F32, tag="trp")
                        nc.tensor.transpose(
                            trp[:], kr2[:, 2 * gg:2 * gg + 2, :], ident[:])
                        ge, go = 2 * gg, 2 * gg + 1
                        nc.vector.tensor_copy(
                            out=kT[0:HD, ge, 0, ts_], in_=trp[0:HD])
                        nc.vector.tensor_copy(
                            out=kT[HD:P, go, 1, ts_], in_=trp[HD:P])
                        # other variant via DMA (partition shift)
                        nc.gpsimd.dma_start(kT[HD:P, ge, 1, ts_], kT[0:HD, ge, 0, ts_])
                        nc.gpsimd.dma_start(kT[0:HD, go, 0, ts_], kT[HD:P, go, 1, ts_])

            # Q projection, flipped: weights stationary, output lands
            # directly in qT layout [outdim-part, S-free]; no PE transposes.
            # RoPE in this layout via a 32-row-block swap copy (DMA) plus
            # full-height elementwise ops with cosT4/sinS patterns. Host
            # permutes wq columns (t0|t1 blocks per head) to match.
            phq = ph1.enter_context(ExitStack())
            wp = phq.enter_context(tc.tile_pool(name="wqpool", bufs=3))
            qrp = phq.enter_context(tc.tile_pool(name="qrope", bufs=3))
            wqTr = wqT.rearrange("(o p) n -> p o n", p=P)
            for jj in range(QH // 2):
                wqb = wp.tile([P, NKD, P], F32R, tag="wqb")
                nc.scalar.dma_start(wqb[:], wqTr[:, :, jj * P:(jj + 1) * P])
                for sh in range(2):
                    ss = slice(sh * (S // 2), (sh + 1) * (S // 2))
                    psq = pp.tile([P, S // 2], F32, tag="qps")
                    for kd in range(NKD):
                        nc.tensor.matmul(
                            psq[:],
                            lhsT=wqb[:, kd, :],
                            rhs=xT_sb[:, kd, ss],
                            start=(kd == 0),
                            stop=(kd == NKD - 1),
                        )
                    qsb = qrp.tile([P, S // 2], F32, tag="qsb")
                    nc.vector.tensor_copy(out=qsb[:], in_=psq[:])
                    qsh = qrp.tile([P, S // 2], F32, tag="qsh")
                    for bb in range(4):
                        sw = 32 if bb % 2 == 0 else -32
                        nc.sync.dma_start(
                            qsh[32 * bb:32 * (bb + 1), :],
                            qsb[32 * bb + sw:32 * (bb + 1) + sw, :])
                    qo = qrp.tile([P, S // 2], F32, tag="qo")
                    qt_ = qrp.tile([P, S // 2], F32, tag="qt_")
                    M = mybir.AluOpType.mult
                    nc.vector.tensor_tensor(
                        out=qo[:], in0=qsb[:], in1=cosT4_sb[:, ss], op=M)
                    nc.vector.tensor_tensor(
                        out=qt_[:], in0=qsh[:], in1=sinS_sb[:, ss], op=M)
                    nc.vector.tensor_add(
                        out=qT[:, jj, ss], in0=qo[:], in1=qt_[:])
            phq.close()

        # ---------------- phase 2: attention --------------------------------
        with ExitStack() as ph2:
            wop = ph2.enter_context(tc.tile_pool(name="wopool", bufs=1))
            atT = wop.tile([P, QH * HD // P, S], F32R)  # attn_out.T blocks
            wos = ph2.enter_context(tc.tile_pool(name="wostream", bufs=2))

            with ExitStack() as ph2i:
                ep = ph2i.enter_context(tc.tile_pool(name="exp", bufs=6))
                lp = ph2i.enter_context(tc.tile_pool(name="lrow", bufs=4))
                sp = ph2i.enter_context(tc.tile_pool(name="scps", bufs=4, space="PSUM"))
                vp = ph2i.enter_context(tc.tile_pool(name="pvps", bufs=4, space="PSUM"))

                def norm(pv, jj, half, qh):
                    # Copy the [65, 512] psum to SBUF right away (cheap, frees
                    # the PSUM slot so the next accumulation can start), then
                    # normalize off the critical path: DMA-reshape the l row
                    # to [128, 4] (reciprocal is ~9cyc/elem and walks the free
                    # dim serially), reshape back, gpsimd-broadcast to 64
                    # partitions, multiply into atT.
                    W = 2 * QC
                    cs = slice(qh * W, (qh + 1) * W)
                    pvs = lp.tile([HD + 1, W], F32, tag="pvs")
                    nc.vector.tensor_copy(out=pvs[:], in_=pv[:])
                    r8 = lp.tile([P, W // P], F32, tag="r8")
                    nc.gpsimd.dma_start(r8[:], pvs[HD:HD + 1, :])
                    nc.vector.reciprocal(out=r8[:], in_=r8[:])
                    rl = lp.tile([1, W], F32, tag="rl")
                    nc.gpsimd.dma_start(rl[:], r8[:])
                    rlb = lp.tile([HD, W], F32, tag="rlb")
                    nc.gpsimd.partition_broadcast(rlb[:], rl[:])
                    if half == 0:
                        nc.vector.tensor_tensor(
                            out=atT[0:HD, jj, cs], in0=pvs[0:HD, :], in1=rlb[:],
                            op=mybir.AluOpType.mult,
                        )
                    else:
                        # reuse rlb in place, then DMA-shift to partitions 64+
                        nc.vector.tensor_tensor(
                            out=rlb[:], in0=pvs[0:HD, :], in1=rlb[:],
                            op=mybir.AluOpType.mult,
                        )
                        nc.gpsimd.dma_start(atT[HD:P, jj, cs].bitcast(F32), rlb[:])

                for jj in range(QH // 2):
                    g = (2 * jj) // NREP
                    for qh in range(2):
                        pvA = vp.tile([HD + 1, 2 * QC], F32, tag="pv")
                        pvB = vp.tile([HD + 1, 2 * QC], F32, tag="pv")
                        for qc in (2 * qh, 2 * qh + 1):
                            qc2 = qc % 2
                            ktmax = 2 * qc + 2 if mode == "causal" else NT
                            for kt in range(ktmax):
                                sc = sp.tile([P, 2, QC], F32, tag="sc")  # 1 bank
                                qs = qT[:, jj, qc * QC:(qc + 1) * QC]
                                nc.tensor.matmul(
                                    sc[:, 0], lhsT=kT[:, g, 0, kt * P:(kt + 1) * P],
                                    rhs=qs, start=True, stop=True)
                                nc.tensor.matmul(
                                    sc[:, 1], lhsT=kT[:, g, 1, kt * P:(kt + 1) * P],
                                    rhs=qs, start=True, stop=True)
                                ex = ep.tile([P, 2, QC], F32R, tag="ex")
                                lo = 0
                                if mode == "causal" and kt == 2 * qc:
                                    nc.vector.tensor_add(
                                        out=sc[:, :, 0:P], in0=sc[:, :, 0:P],
                                        in1=tri_sb[:, None, :].to_broadcast((P, 2, P)))
                                if mode == "causal" and kt == 2 * qc + 1:
                                    nc.vector.tensor_add(
                                        out=sc[:, :, P:QC], in0=sc[:, :, P:QC],
                                        in1=tri_sb[:, None, :].to_broadcast((P, 2, P)))
                                    nc.vector.memset(ex[:, :, 0:P].bitcast(F32), 0.0)
                                    lo = P
                                if mode == "mask":
                                    nc.vector.tensor_add(
                                        out=sc[:], in0=sc[:],
                                        in1=maskT_sb[:, kt, None, qc * QC:(qc + 1) * QC]
                                        .to_broadcast((P, 2, QC)))
                                nc.scalar.activation(
                                    out=ex[:, :, lo:QC],
                                    in_=sc[:, :, lo:QC],
                                    func=mybir.ActivationFunctionType.Exp,
                                    scale=float(SCALE),
                                )
                                nc.tensor.matmul(
                                    pvA[:, qc2 * QC:(qc2 + 1) * QC],
                                    lhsT=vsb[:, kt, g, :], rhs=ex[:, 0],
                                    start=(kt == 0), stop=(kt == ktmax - 1))
                                nc.tensor.matmul(
                                    pvB[:, qc2 * QC:(qc2 + 1) * QC],
                                    lhsT=vsb[:, kt, g, :], rhs=ex[:, 1],
                                    start=(kt == 0), stop=(kt == ktmax - 1))
                        norm(pvA, jj, 0, qh)
                        norm(pvB, jj, 1, qh)

            # ---------------- phase 3: output projection --------------------
            with ExitStack() as ph3:
                op_ = ph3.enter_context(tc.tile_pool(name="ops", bufs=3, space="PSUM"))
                ob_ = ph3.enter_context(tc.tile_pool(name="obounce", bufs=3))
                NB = QH * HD // P  # 8 blocks
                woTr = woT.rearrange("(j p) d -> p j d", p=P)
                for dc in range(D // 512):
                    wo_sb = wos.tile([P, NB, 512], F32R, tag="wochunk")
                    nc.scalar.dma_start(
                        wo_sb[:], woTr[:, :, dc * 512:(dc + 1) * 512])
                    for t in range(NT):
                        po = op_.tile([P, 512], F32, tag="po")
                        for jj in range(NB):
                            nc.tensor.matmul(
                                po[:],
                                lhsT=atT[:, jj, t * P:(t + 1) * P],
                                rhs=wo_sb[:, jj, :],
                                start=(jj == 0),
                                stop=(jj == NB - 1),
                            )
                        pos = ob_.tile([P, 512], F32, tag="pos")
                        nc.vector.tensor_copy(out=pos[:], in_=po[:])
                        nc.sync.dma_start(
                            outp[t * P:(t + 1) * P, dc * 512:(dc + 1) * 512], pos[:]
                        )

    nc.compile()
    return nc


_NC_CACHE = {}


def _get_nc(mode):
    if mode not in _NC_CACHE:
        _NC_CACHE[mode] = build_nc(mode)
    return _NC_CACHE[mode]


def detect_mode(mask):
    m = np.asarray(mask).reshape(S, S)
    if not np.any(m):
        return "zeros"
    tril = np.tril(np.ones((S, S), dtype=bool))
    if np.all(m[tril] == 0.0) and np.all(m[~tril] <= -1e30):
        return "causal"
    return "mask"


def make_in_maps(x, freqs, mask, wq, wk, wv, wo, mode):
    x = np.ascontiguousarray(np.asarray(x, dtype=np.float32))
    wq = np.asarray(wq, dtype=np.float32)
    wk = np.asarray(wk, dtype=np.float32)
    wv = np.asarray(wv, dtype=np.float32)
    wo = np.asarray(wo, dtype=np.float32)
    freqs = np.asarray(freqs, dtype=np.float32)
    cosf = np.cos(freqs).astype(np.float32)
    sinf = np.sin(freqs).astype(np.float32)
    cosT4 = np.ascontiguousarray(np.tile(cosf.T, (4, 1)))        # (128, S)
    sinT = sinf.T                                                # (32, S)
    sinS = np.ascontiguousarray(
        np.concatenate([-sinT, sinT, -sinT, sinT], axis=0))      # (128, S)
    # within-head deinterleave permutation for q: t0 block then t1 block
    hperm = np.concatenate([np.arange(0, HD, 2), np.arange(1, HD, 2)])
    qperm = np.concatenate([h * HD + hperm for h in range(QH)])

    HW = QH * HD  # 1024 head dims per group
    in_maps = []
    xT = [np.ascontiguousarray(x[b].T) for b in range(B)]
    for core in range(8):
        b, hg = core // 2, core % 2
        wq_g = wq[hg * HW:(hg + 1) * HW]
        wk_g = wk[hg * KVH * HD:(hg + 1) * KVH * HD]
        wv_g = wv[hg * KVH * HD:(hg + 1) * KVH * HD]
        wkvT = np.ascontiguousarray(
            np.concatenate([wk_g.T, wv_g.T], axis=1))
        m = {
            "xT": xT[b],
            "wqT": np.ascontiguousarray(wq_g.T[:, qperm]),
            "wkvT": wkvT,
            "woT": np.ascontiguousarray(wo[:, hg * HW:(hg + 1) * HW].T),
            "cosf": cosf,
            "sinf": sinf,
            "cosT4": cosT4,
            "sinS": sinS,
        }
        mm = np.asarray(mask, dtype=np.float32).reshape(S, S)
        if mode == "causal":
            tm = mm[:P, :P].T.astype(np.float64) / SCALE
            m["trimask"] = np.clip(tm, -3.0e38, 3.0e38).astype(np.float32)
        if mode == "mask":
            mt = mm.T.astype(np.float64) / SCALE
            m["maskT"] = np.ascontiguousarray(
                np.clip(mt, -3.0e38, 3.0e38).astype(np.float32))
        in_maps.append(m)
    return in_maps


def assemble(results):
    out = np.empty((B, S, D), dtype=np.float32)
    k = np.empty((B, NH, S, HD), dtype=np.float32)
    v = np.empty((B, NH, S, HD), dtype=np.float32)
    for core in range(8):
        b, hg = core // 2, core % 2
        r = results[core]
        if hg == 0:
            out[b] = r["outp"]
        else:
            out[b] += r["outp"]
        k[b, hg * QH:(hg + 1) * QH] = np.repeat(r["ko"], NREP, axis=0)
        v[b, hg * QH:(hg + 1) * QH] = np.repeat(r["vo"], NREP, axis=0)
    return out, k, v


def kernel(x, freqs, mask, wq, wk, wv, wo):
    mode = detect_mode(mask)
    nc = _get_nc(mode)
    in_maps = make_in_maps(x, freqs, mask, wq, wk, wv, wo, mode)
    res = run_bass_kernel_spmd(nc, in_maps, core_ids=list(range(8)))
    return assemble(res.results)


# revision 29
# speedup vs baseline: 1.0522x; 1.0522x over previous
"""Trainium2 Bass kernel for a GQA attention block (B=4, S=1024, D=2048,
NH=32, NKV=8, HD=64) with RoPE, causal mask, and output projection.

Sharding: 8 cores = 4 batches x 2 head-groups (tensor-parallel over heads).
Each core computes, for its (batch b, head-group hg):
  q/k/v projections (contracted over D), RoPE, attention for 16 q-heads /
  4 kv-heads, and a partial out = attn_out @ wo[:, hg].T.
Host sums the two partial outputs per batch and assembles the (repeated)
k/v caches from the per-core unique kv heads.

All matmuls run as float32r (fp32 storage, fast PE path; ~1cyc/row at
K=128). Attention uses the transposed-scores layout scores_T[k, q]:
softmax normalization (sum over k) falls out of the PV matmul via an
extra ones-column in V, and exp() is a single ACT pass PSUM->SBUF.
Scores contract over HD=64 only, which is LDWEIGHTS-bound at K=64, so
kT stores each kv head zero-padded to K=128 in two half-aligned
variants and both heads of a pair share one [128, 512] scores psum /
exp tile (halves the ACT instruction count).
"""

import numpy as np
from contextlib import ExitStack

import concourse.bass as bass
import concourse.mybir as mybir
import concourse.tile as tile
from concourse import bacc
from concourse.bass_utils import run_bass_kernel_spmd
from concourse.masks import make_identity

F32 = mybir.dt.float32
F32R = mybir.dt.float32r

B, S, D = 4, 1024, 2048
NH, NKV, HD = 32, 8, 64
NREP = NH // NKV
P = 128
NT = S // P            # 8 s-tiles of 128
QH = NH // 2           # 16 q heads per core
KVH = NKV // 2         # 4 kv heads per core
NKD = D // P           # 16 contraction tiles over D
QCH = 4                # q chunks of 256 for attention
QC = S // QCH          # 256
SCALE = 1.0 / np.sqrt(HD)  # 0.125


def build_nc(mode):
    """mode: 'causal' (block-skip + diag tri mask), 'zeros' (dense, no mask),
    'mask' (dense + full additive mask)."""
    nc = bacc.Bacc("TRN2", target_bir_lowering=False, debug=False)

    xT = nc.dram_tensor("xT", [D, S], F32R, kind="ExternalInput")
    wqT = nc.dram_tensor("wqT", [D, QH * HD], F32R, kind="ExternalInput")
    wkvT = nc.dram_tensor("wkvT", [D, 2 * KVH * HD], F32R, kind="ExternalInput")
    woT = nc.dram_tensor("woT", [QH * HD, D], F32R, kind="ExternalInput")
    cosf = nc.dram_tensor("cosf", [S, HD // 2], F32, kind="ExternalInput")
    sinf = nc.dram_tensor("sinf", [S, HD // 2], F32, kind="ExternalInput")
    # q-rope runs in [hd-part, S-free] layout: cos replicated on all four
    # 32-row blocks; sin sign-flipped on t0 blocks (o = q*cos + qswap*sinS)
    cosT4 = nc.dram_tensor("cosT4", [P, S], F32, kind="ExternalInput")
    sinS = nc.dram_tensor("sinS", [P, S], F32, kind="ExternalInput")
    if mode == "causal":
        trimask = nc.dram_tensor("trimask", [P, P], F32, kind="ExternalInput")
    if mode == "mask":
        maskT = nc.dram_tensor("maskT", [S, S], F32, kind="ExternalInput")

    outp = nc.dram_tensor("outp", [S, D], F32, kind="ExternalOutput")
    ko = nc.dram_tensor("ko", [KVH, S, HD], F32, kind="ExternalOutput")
    vo = nc.dram_tensor("vo", [KVH, S, HD], F32, kind="ExternalOutput")

    with tile.TileContext(nc) as tc, ExitStack() as top:
        constp = top.enter_context(tc.tile_pool(name="const", bufs=1))
        ident = constp.tile([P, P], F32)
        make_identity(nc, ident[:])
        cos_sb = constp.tile([P, NT, HD // 2], F32)
        sin_sb = constp.tile([P, NT, HD // 2], F32)
        nc.sync.dma_start(cos_sb[:], cosf.rearrange("(t p) f -> p t f", p=P))
        nc.sync.dma_start(sin_sb[:], sinf.rearrange("(t p) f -> p t f", p=P))
        cosT4_sb = constp.tile([P, S], F32)
        sinS_sb = constp.tile([P, S], F32)
        nc.sync.dma_start(cosT4_sb[:], cosT4[:])
        nc.sync.dma_start(sinS_sb[:], sinS[:])
        if mode == "causal":
            tri_sb = constp.tile([P, P], F32)
            nc.sync.dma_start(tri_sb[:], trimask[:])
        if mode == "mask":
            maskT_sb = constp.tile([P, NT, S], F32)
            nc.sync.dma_start(maskT_sb[:], maskT.rearrange("(t p) q -> p t q", p=P))

        # Single PSUM pool for the whole kernel: every psum tile here fits
        # one bank, so two 4-slot tags cover proj/attn/wo with NO pool
        # transitions (pool swaps drain the PE and re-trip the HAM throttle).
        psp = top.enter_context(tc.tile_pool(name="psall", bufs=4, space="PSUM"))

        # persistent across phases: qT/kT (transposed, rope'd), v (+ones col)
        persist = top.enter_context(tc.tile_pool(name="persist", bufs=1))
        qT = persist.tile([P, QH // 2, S], F32R)      # block jj: heads 2jj,2jj+1
        # kT holds each kv head zero-padded to K=128 in two variants:
        # variant 0 = [k_g | 0] pairs with the q-half on partitions 0:64,
        # variant 1 = [0 | k_g] pairs with the q-half on partitions 64:128.
        # This lets every scores matmul run with full-128 contraction (the
        # K=64 form is ~2x slower, LDWEIGHTS-bound) and full-partition APs.
        kT = persist.tile([P, KVH, 2, S], F32R)
        nc.vector.memset(kT[:].bitcast(F32), 0.0)
        vsb = persist.tile([P, NT, KVH, HD + 1], F32R)
        nc.vector.memset(vsb[:, :, :, HD].bitcast(F32), 1.0)

        # ---------------- phase 1: projections + rope + transposes ----------
        with ExitStack() as ph1:
            xp = ph1.enter_context(tc.tile_pool(name="xpool", bufs=1))
            rp = ph1.enter_context(tc.tile_pool(name="rope", bufs=3))

            xT_sb = xp.tile([P, NKD, S], F32R)
            xTr = xT.rearrange("(o p) s -> p o s", p=P)
            for xq4 in range(4):
                nc.sync.dma_start(
                    xT_sb[:, 4 * xq4:4 * (xq4 + 1), :],
                    xTr[:, 4 * xq4:4 * (xq4 + 1), :])

            def rope(dst, src, t, nh):
                """dst, src: [P, nh, HD] APs (dst SBUF, src PSUM), s-tile t."""
                t0 = src[:, :, 0::2]
                t1 = src[:, :, 1::2]
                r0 = dst[:, :, 0::2]
                r1 = dst[:, :, 1::2]
                cosb = cos_sb[:, t, None, :].to_broadcast((P, nh, HD // 2))
                sinb = sin_sb[:, t, None, :].to_broadcast((P, nh, HD // 2))
                tmp = rp.tile([P, nh, HD // 2], F32, tag=f"ropetmp{nh}")
                M = mybir.AluOpType.mult
                nc.vector.tensor_tensor(out=tmp[:], in0=t1, in1=sinb, op=M)
                nc.vector.tensor_tensor(out=r0, in0=t0, in1=cosb, op=M)
                nc.vector.tensor_sub(out=r0, in0=r0, in1=tmp[:])
                nc.vector.tensor_tensor(out=tmp[:], in0=t0, in1=sinb, op=M)
                nc.vector.tensor_tensor(out=r1, in0=t1, in1=cosb, op=M)
                nc.vector.tensor_add(out=r1, in0=r1, in1=tmp[:])

            # K/V projection (k cols 0:256, v cols 256:512 of wkv)
            with ExitStack() as phkv:
                wkvp = phkv.enter_context(tc.tile_pool(name="wkvpool", bufs=1))
                wkv_sb = wkvp.tile([P, NKD, 2 * KVH * HD], F32R)
                nc.scalar.dma_start(wkv_sb[:], wkvT.rearrange("(o p) n -> p o n", p=P))
                for t in range(NT):
                    ps = psp.tile([P, 2, KVH, HD], F32, tag="acc")
                    for kd in range(NKD):
                        nc.tensor.matmul(
                            ps[:],
                            lhsT=xT_sb[:, kd, t * P:(t + 1) * P],
                            rhs=wkv_sb[:, kd, :],
                            start=(kd == 0),
                            stop=(kd == NKD - 1),
                        )
                    # V: copy into vsb (ones col pre-set), DMA out
                    nc.vector.tensor_copy(out=vsb[:, t, :, 0:HD], in_=ps[:, 1])
                    for g in range(KVH):
                        nc.gpsimd.dma_start(
                            vo[g, t * P:(t + 1) * P, :], vsb[:, t, g, 0:HD].bitcast(F32))
                    # K: rope, DMA out, transpose into kT
                    kr = rp.tile([P, KVH, HD], F32, tag="krope")
                    rope(kr[:], ps[:, 0], t, KVH)
                    for g in range(KVH):
                        nc.gpsimd.dma_start(ko[g, t * P:(t + 1) * P, :], kr[:, g, :])
                    ts_ = slice(t * P, (t + 1) * P)
                    # deinterleave hd pairs into t0|t1 blocks to match the
                    # permuted q layout (scores are permutation-invariant)
                    kr2 = rp.tile([P, KVH, HD], F32, tag="krope2")
                    nc.vector.tensor_copy(
                        out=kr2[:, :, 0:HD // 2], in_=kr[:, :, 0::2])
                    nc.vector.tensor_copy(
                        out=kr2[:, :, HD // 2:HD], in_=kr[:, :, 1::2])
                    for gg in range(KVH // 2):
                        trp = psp.tile([P, P], F32, tag="sc")
                        nc.tensor.transpose(
                            trp[:], kr2[:, 2 * gg:2 * gg + 2, :], ident[:])
                        ge, go = 2 * gg, 2 * gg + 1
                        nc.vector.tensor_copy(
                            out=kT[0:HD, ge, 0, ts_], in_=trp[0:HD])
                        nc.vector.tensor_copy(
                            out=kT[HD:P, go, 1, ts_], in_=trp[HD:P])
                        # other variant via DMA (partition shift)
                        nc.gpsimd.dma_start(kT[HD:P, ge, 1, ts_], kT[0:HD, ge, 0, ts_])
                        nc.gpsimd.dma_start(kT[0:HD, go, 0, ts_], kT[HD:P, go, 1, ts_])

            # Q projection, flipped: weights stationary, output lands
            # directly in qT layout [outdim-part, S-free]; no PE transposes.
            # RoPE in this layout via a 32-row-block swap copy (DMA) plus
            # full-height elementwise ops with cosT4/sinS patterns. Host
            # permutes wq columns (t0|t1 blocks per head) to match.
            phq = ph1.enter_context(ExitStack())
            wp = phq.enter_context(tc.tile_pool(name="wqpool", bufs=3))
            qrp = phq.enter_context(tc.tile_pool(name="qrope", bufs=3))
            wqTr = wqT.rearrange("(o p) n -> p o n", p=P)
            for jj in range(QH // 2):
                wqb = wp.tile([P, NKD, P], F32R, tag="wqb")
                nc.scalar.dma_start(wqb[:], wqTr[:, :, jj * P:(jj + 1) * P])
                for sh in range(2):
                    ss = slice(sh * (S // 2), (sh + 1) * (S // 2))
                    psq = psp.tile([P, S // 2], F32, tag="acc")
                    for kd in range(NKD):
                        nc.tensor.matmul(
                            psq[:],
                            lhsT=wqb[:, kd, :],
                            rhs=xT_sb[:, kd, ss],
                            start=(kd == 0),
                            stop=(kd == NKD - 1),
                        )
                    qsb = qrp.tile([P, S // 2], F32, tag="qsb")
                    nc.vector.tensor_copy(out=qsb[:], in_=psq[:])
                    qsh = qrp.tile([P, S // 2], F32, tag="qsh")
                    for bb in range(4):
                        sw = 32 if bb % 2 == 0 else -32
                        nc.sync.dma_start(
                            qsh[32 * bb:32 * (bb + 1), :],
                            qsb[32 * bb + sw:32 * (bb + 1) + sw, :])
                    qo = qrp.tile([P, S // 2], F32, tag="qo")
                    qt_ = qrp.tile([P, S // 2], F32, tag="qt_")
                    M = mybir.AluOpType.mult
                    nc.vector.tensor_tensor(
                        out=qo[:], in0=qsb[:], in1=cosT4_sb[:, ss], op=M)
                    nc.vector.tensor_tensor(
                        out=qt_[:], in0=qsh[:], in1=sinS_sb[:, ss], op=M)
                    nc.vector.tensor_add(
                        out=qT[:, jj, ss], in0=qo[:], in1=qt_[:])
            phq.close()

        # ---------------- phase 2: attention --------------------------------
        with ExitStack() as ph2:
            wop = ph2.enter_context(tc.tile_pool(name="wopool", bufs=1))
            atT = wop.tile([P, QH * HD // P, S], F32R)  # attn_out.T blocks
            wos = ph2.enter_context(tc.tile_pool(name="wostream", bufs=2))

            with ExitStack() as ph2i:
                ep = ph2i.enter_context(tc.tile_pool(name="exp", bufs=6))
                lp = ph2i.enter_context(tc.tile_pool(name="lrow", bufs=4))

                def norm(pv, jj, half, qh):
                    # Copy the [65, 512] psum to SBUF right away (cheap, frees
                    # the PSUM slot so the next accumulation can start), then
                    # normalize off the critical path: DMA-reshape the l row
                    # to [128, 4] (reciprocal is ~9cyc/elem and walks the free
                    # dim serially), reshape back, gpsimd-broadcast to 64
                    # partitions, multiply into atT.
                    W = 2 * QC
                    cs = slice(qh * W, (qh + 1) * W)
                    pvs = lp.tile([HD + 1, W], F32, tag="pvs")
                    nc.vector.tensor_copy(out=pvs[:], in_=pv[:])
                    r8 = lp.tile([P, W // P], F32, tag="r8")
                    nc.gpsimd.dma_start(r8[:], pvs[HD:HD + 1, :])
                    nc.vector.reciprocal(out=r8[:], in_=r8[:])
                    rl = lp.tile([1, W], F32, tag="rl")
                    nc.gpsimd.dma_start(rl[:], r8[:])
                    rlb = lp.tile([HD, W], F32, tag="rlb")
                    nc.gpsimd.partition_broadcast(rlb[:], rl[:])
                    if half == 0:
                        nc.vector.tensor_tensor(
                            out=atT[0:HD, jj, cs], in0=pvs[0:HD, :], in1=rlb[:],
                            op=mybir.AluOpType.mult,
                        )
                    else:
                        # reuse rlb in place, then DMA-shift to partitions 64+
                        nc.vector.tensor_tensor(
                            out=rlb[:], in0=pvs[0:HD, :], in1=rlb[:],
                            op=mybir.AluOpType.mult,
                        )
                        nc.gpsimd.dma_start(atT[HD:P, jj, cs].bitcast(F32), rlb[:])

                for jj in range(QH // 2):
                    g = (2 * jj) // NREP
                    for qh in range(2):
                        pvA = psp.tile([HD + 1, 2 * QC], F32, tag="acc")
                        pvB = psp.tile([HD + 1, 2 * QC], F32, tag="acc")
                        for qc in (2 * qh, 2 * qh + 1):
                            qc2 = qc % 2
                            ktmax = 2 * qc + 2 if mode == "causal" else NT
                            for kt in range(ktmax):
                                sc = psp.tile([P, 2, QC], F32, tag="sc")  # 1 bank
                                qs = qT[:, jj, qc * QC:(qc + 1) * QC]
                                nc.tensor.matmul(
                                    sc[:, 0], lhsT=kT[:, g, 0, kt * P:(kt + 1) * P],
                                    rhs=qs, start=True, stop=True)
                                nc.tensor.matmul(
                                    sc[:, 1], lhsT=kT[:, g, 1, kt * P:(kt + 1) * P],
                                    rhs=qs, start=True, stop=True)
                                ex = ep.tile([P, 2, QC], F32R, tag="ex")
                                lo = 0
                                if mode == "causal" and kt == 2 * qc:
                                    nc.vector.tensor_add(
                                        out=sc[:, :, 0:P], in0=sc[:, :, 0:P],
                                        in1=tri_sb[:, None, :].to_broadcast((P, 2, P)))
                                if mode == "causal" and kt == 2 * qc + 1:
                                    nc.vector.tensor_add(
                                        out=sc[:, :, P:QC], in0=sc[:, :, P:QC],
                                        in1=tri_sb[:, None, :].to_broadcast((P, 2, P)))
                                    nc.vector.memset(ex[:, :, 0:P].bitcast(F32), 0.0)
                                    lo = P
                                if mode == "mask":
                                    nc.vector.tensor_add(
                                        out=sc[:], in0=sc[:],
                                        in1=maskT_sb[:, kt, None, qc * QC:(qc + 1) * QC]
                                        .to_broadcast((P, 2, QC)))
                                nc.scalar.activation(
                                    out=ex[:, :, lo:QC],
                                    in_=sc[:, :, lo:QC],
                                    func=mybir.ActivationFunctionType.Exp,
                                    scale=float(SCALE),
                                )
                                nc.tensor.matmul(
                                    pvA[:, qc2 * QC:(qc2 + 1) * QC],
                                    lhsT=vsb[:, kt, g, :], rhs=ex[:, 0],
                                    start=(kt == 0), stop=(kt == ktmax - 1))
                                nc.tensor.matmul(
                                    pvB[:, qc2 * QC:(qc2 + 1) * QC],
                                    lhsT=vsb[:, kt, g, :], rhs=ex[:, 1],
                                    start=(kt == 0), stop=(kt == ktmax - 1))
                        norm(pvA, jj, 0, qh)
                        norm(pvB, jj, 1, qh)

            # ---------------- phase 3: output projection --------------------
            with ExitStack() as ph3:
                ob_ = ph3.enter_context(tc.tile_pool(name="obounce", bufs=3))
                NB = QH * HD // P  # 8 blocks
                woTr = woT.rearrange("(j p) d -> p j d", p=P)
                for dc in range(D // 512):
                    wo_sb = wos.tile([P, NB, 512], F32R, tag="wochunk")
                    nc.scalar.dma_start(
                        wo_sb[:], woTr[:, :, dc * 512:(dc + 1) * 512])
                    for t in range(NT):
                        po = psp.tile([P, 512], F32, tag="acc")
                        for jj in range(NB):
                            nc.tensor.matmul(
                                po[:],
                                lhsT=atT[:, jj, t * P:(t + 1) * P],
                                rhs=wo_sb[:, jj, :],
                                start=(jj == 0),
                                stop=(jj == NB - 1),
                            )
                        pos = ob_.tile([P, 512], F32, tag="pos")
                        nc.vector.tensor_copy(out=pos[:], in_=po[:])
                        nc.sync.dma_start(
                            outp[t * P:(t + 1) * P, dc * 512:(dc + 1) * 512], pos[:]
                        )

    nc.compile()
    return nc


_NC_CACHE = {}


def _get_nc(mode):
    if mode not in _NC_CACHE:
        _NC_CACHE[mode] = build_nc(mode)
    return _NC_CACHE[mode]


def detect_mode(mask):
    m = np.asarray(mask).reshape(S, S)
    if not np.any(m):
        return "zeros"
    tril = np.tril(np.ones((S, S), dtype=bool))
    if np.all(m[tril] == 0.0) and np.all(m[~tril] <= -1e30):
        return "causal"
    return "mask"


def make_in_maps(x, freqs, mask, wq, wk, wv, wo, mode):
    x = np.ascontiguousarray(np.asarray(x, dtype=np.float32))
    wq = np.asarray(wq, dtype=np.float32)
    wk = np.asarray(wk, dtype=np.float32)
    wv = np.asarray(wv, dtype=np.float32)
    wo = np.asarray(wo, dtype=np.float32)
    freqs = np.asarray(freqs, dtype=np.float32)
    cosf = np.cos(freqs).astype(np.float32)
    sinf = np.sin(freqs).astype(np.float32)
    cosT4 = np.ascontiguousarray(np.tile(cosf.T, (4, 1)))        # (128, S)
    sinT = sinf.T                                                # (32, S)
    sinS = np.ascontiguousarray(
        np.concatenate([-sinT, sinT, -sinT, sinT], axis=0))      # (128, S)
    # within-head deinterleave permutation for q: t0 block then t1 block
    hperm = np.concatenate([np.arange(0, HD, 2), np.arange(1, HD, 2)])
    qperm = np.concatenate([h * HD + hperm for h in range(QH)])

    HW = QH * HD  # 1024 head dims per group
    in_maps = []
    xT = [np.ascontiguousarray(x[b].T) for b in range(B)]
    for core in range(8):
        b, hg = core // 2, core % 2
        wq_g = wq[hg * HW:(hg + 1) * HW]
        wk_g = wk[hg * KVH * HD:(hg + 1) * KVH * HD]
        wv_g = wv[hg * KVH * HD:(hg + 1) * KVH * HD]
        wkvT = np.ascontiguousarray(
            np.concatenate([wk_g.T, wv_g.T], axis=1))
        m = {
            "xT": xT[b],
            "wqT": np.ascontiguousarray(wq_g.T[:, qperm]),
            "wkvT": wkvT,
            "woT": np.ascontiguousarray(wo[:, hg * HW:(hg + 1) * HW].T),
            "cosf": cosf,
            "sinf": sinf,
            "cosT4": cosT4,
            "sinS": sinS,
        }
        mm = np.asarray(mask, dtype=np.float32).reshape(S, S)
        if mode == "causal":
            tm = mm[:P, :P].T.astype(np.float64) / SCALE
            m["trimask"] = np.clip(tm, -3.0e38, 3.0e38).astype(np.float32)
        if mode == "mask":
            mt = mm.T.astype(np.float64) / SCALE
            m["maskT"] = np.ascontiguousarray(
                np.clip(mt, -3.0e38, 3.0e38).astype(np.float32))
        in_maps.append(m)
    return in_maps


def assemble(results):
    out = np.empty((B, S, D), dtype=np.float32)
    k = np.empty((B, NH, S, HD), dtype=np.float32)
    v = np.empty((B, NH, S, HD), dtype=np.float32)
    for core in range(8):
        b, hg = core // 2, core % 2
        r = results[core]
        if hg == 0:
            out[b] = r["outp"]
        else:
            out[b] += r["outp"]
        k[b, hg * QH:(hg + 1) * QH] = np.repeat(r["ko"], NREP, axis=0)
        v[b, hg * QH:(hg + 1) * QH] = np.repeat(r["vo"], NREP, axis=0)
    return out, k, v


def kernel(x, freqs, mask, wq, wk, wv, wo):
    mode = detect_mode(mask)
    nc = _get_nc(mode)
    in_maps = make_in_maps(x, freqs, mask, wq, wk, wv, wo, mode)
    res = run_bass_kernel_spmd(nc, in_maps, core_ids=list(range(8)))
    return assemble(res.results)
